# revision 30
# baseline (speedup 1.0000x reference)
"""Multi-head attention (B=4, S=2048, D=1024, H=16) on 8 trn2 NeuronCores.

Sharding: core = (batch b, head-group g) with b = core//2, g = core%2.
Each core handles one batch and 8 heads (512 of the 1024 d_model dims):
  - host pre-tiles query/key/value[b] and the weight slices into the exact
    SBUF layouts ([sc, 128, a, s] slabs / [128, a, d] weights) so every DMA
    is a contiguous 8KB-per-partition transfer (no strided descriptors)
  - device computes Q^T, K^T (head dims on partitions) and V (natural),
    attention with *transposed* scores S^T = K_h @ Q_h^T so softmax's
    denominator comes out of the PV matmul via a ones-column appended to V
  - output projection vs Wo[g*512:(g+1)*512, :] gives a partial [2048,1024]
  - host sums the two group partials per batch and adds bv@Wo + bo
Matmul operand dtype is MM_DT (bf16 default). PSUM accumulation and the
softmax normalization chain stay fp32.

v3 scheduling: projection/output-projection work is fed to the PE in 4-MM
chunks from a global filler queue, one chunk per odd kt slot, which fits
the ~1us/2kt PE slack under the ACT-bound exp stream (whole 8-MM groups
stalled the exp pipeline ~2.1us each). Prologue DMAs are ordered per queue
(gpsimd: wk,wq,wv / sync: xs k0,q0,q1 / scalar: xv0,wo) so the first
scores inputs arrive first.
"""

import os
import numpy as np
from contextlib import ExitStack

B = 4
S = 2048
D = 1024
H = 16
DK = 64
NCORES = 8
GH = 8          # heads per core (group)
GD = GH * DK    # 512 head dims per core
NCH = GD // 128  # 4 chunks of 128 output dims
KT = S // 128    # 16 key tiles
QC = 1024        # q chunk width for attention
NQC = S // QC    # 2
SC = 512         # s chunk width for projections
NSC = S // SC    # 4
DMT = D // 128   # 8 d_model tiles

MM_DT = os.environ.get("MM_DT", "bf16")  # "bf16" | "f32r"

_CACHE = {}


def _np_mm_dtype():
    if MM_DT == "bf16":
        import ml_dtypes
        return ml_dtypes.bfloat16
    return np.float32


def _build_program():
    import concourse.mybir as mybir
    import concourse.tile as tile
    from concourse import bacc
    from concourse.tile_rust import add_dep_helper

    f32 = mybir.dt.float32
    dmm = mybir.dt.bfloat16 if MM_DT == "bf16" else mybir.dt.float32r

    nc = bacc.Bacc("TRN2", target_bir_lowering=False, debug=False,
                   num_devices=NCORES)

    xq = nc.dram_tensor("xq", [NSC, 128, DMT, SC], dmm,
                        kind="ExternalInput").ap()
    xk = nc.dram_tensor("xk", [NSC, 128, DMT, SC], dmm,
                        kind="ExternalInput").ap()
    xv = nc.dram_tensor("xv", [NSC, 128, DMT, SC], dmm,
                        kind="ExternalInput").ap()
    wq = nc.dram_tensor("wq", [128, DMT, GD], dmm, kind="ExternalInput").ap()
    wk = nc.dram_tensor("wk", [128, DMT, GD], dmm, kind="ExternalInput").ap()
    wv = nc.dram_tensor("wv", [128, DMT, GD], dmm, kind="ExternalInput").ap()
    wo = nc.dram_tensor("wo", [128, NCH, D], dmm, kind="ExternalInput").ap()
    bq = nc.dram_tensor("bq", [128, NCH], f32, kind="ExternalInput").ap()
    bk = nc.dram_tensor("bk", [128, NCH], f32, kind="ExternalInput").ap()
    out = nc.dram_tensor("out", [S, D], f32, kind="ExternalOutput").ap()

    Exp = mybir.ActivationFunctionType.Exp

    with tile.TileContext(nc) as tc, ExitStack() as ctx:
        # ---- pools (slots are statically reserved per tag) ----
        p_qt = ctx.enter_context(tc.tile_pool(name="qt", bufs=GH))
        p_kt = ctx.enter_context(tc.tile_pool(name="kt", bufs=GH))
        p_v = ctx.enter_context(tc.tile_pool(name="v", bufs=KT))
        p_ot = ctx.enter_context(tc.tile_pool(name="ot", bufs=NCH))
        p_wvo = ctx.enter_context(tc.tile_pool(name="wvo", bufs=1))
        p_wc = ctx.enter_context(tc.tile_pool(name="wc", bufs=1))
        p_bias = ctx.enter_context(tc.tile_pool(name="bias", bufs=1))
        p_xs = ctx.enter_context(tc.tile_pool(name="xs", bufs=4))
        p_pt = ctx.enter_context(tc.tile_pool(name="pt", bufs=7))
        p_zr = ctx.enter_context(tc.tile_pool(name="zr", bufs=2))
        p_rb = ctx.enter_context(tc.tile_pool(name="rb", bufs=2))
        p_st = ctx.enter_context(tc.tile_pool(name="st", bufs=2))
        p_ov = ctx.enter_context(tc.tile_pool(name="ov", bufs=2))
        # PSUM: ps 3 slots x 2 banks + pv 1 slot x 2 banks = all 8 banks
        p_ps = ctx.enter_context(tc.tile_pool(name="ps", bufs=3, space="PSUM"))
        p_pv = ctx.enter_context(tc.tile_pool(name="pv", bufs=1, space="PSUM"))

        # ---- biases + ones (tiny, sync queue) ----
        bq_sb = p_bias.tile([128, NCH], f32, tag="bq")
        nc.sync.dma_start(out=bq_sb[:], in_=bq)
        bk_sb = p_bias.tile([128, NCH], f32, tag="bk")
        nc.sync.dma_start(out=bk_sb[:], in_=bk)
        ones_sb = p_bias.tile([128, 1], f32, tag="ones")
        nc.vector.memset(ones_sb[:], 1.0)

        # ---- prologue DMAs in two dependency-gated waves: the DMA engines
        # fair-share HBM bandwidth across everything in flight, so gate the
        # non-critical wave behind the 5MB the first scores matmul needs
        # (wk, wq, xs k0/q0/q1). V arrives in wave2; the exp stream runs
        # ~7 kt ahead of PV, which absorbs the later V start.
        wk_sb = p_wc.tile([128, DMT, GD], dmm, tag="wkc", name="wk_sb")
        i_wk = nc.gpsimd.dma_start(out=wk_sb[:], in_=wk)
        wq_sb = p_wc.tile([128, DMT, GD], dmm, tag="wqc", name="wq_sb")
        i_wq = nc.gpsimd.dma_start(out=wq_sb[:], in_=wq)
        xs_k0 = p_xs.tile([128, DMT, SC], dmm, tag="xs", name="xs_k0")
        i_k0 = nc.sync.dma_start(out=xs_k0[:], in_=xk[0])
        xs_q0 = p_xs.tile([128, DMT, SC], dmm, tag="xs", name="xs_q0")
        i_q0 = nc.sync.dma_start(out=xs_q0[:], in_=xq[0])
        xs_q1 = p_xs.tile([128, DMT, SC], dmm, tag="xs", name="xs_q1")
        i_q1 = nc.scalar.dma_start(out=xs_q1[:], in_=xq[1])
        wave1 = [i_wk, i_wq, i_k0, i_q0, i_q1]

        xv_0 = p_xs.tile([128, DMT, 512], dmm, tag="xs", name="xv0")
        i_xv0 = nc.scalar.dma_start(out=xv_0[:], in_=xv[0])
        vstate = {"pre": xv_0}
        wv_sb = p_wvo.tile([128, DMT, GD], dmm, tag="wvo", name="wv_sb")
        i_wv = nc.gpsimd.dma_start(out=wv_sb[:], in_=wv)
        wave1b = [i_xv0, i_wv]

        wo_sb = p_wvo.tile([128, NCH, D], dmm, tag="wo", name="wo_sb")
        i_wo = nc.scalar.dma_start(out=wo_sb[:], in_=wo)
        wave2 = [i_wo]

        for later, earlier in (
                [(b, a) for b in wave1b for a in wave1]
                + [(w, b) for w in wave2 for b in wave1b]):
            add_dep_helper(later.ins, earlier.ins, reason="dma wave order")

        qt_sb = [None] * GH
        kt_sb = [None] * GH
        ot_sb = [None] * NCH
        v_sb = []

        # ---- V projection: V_sb[st] = [128 s, GH, 65] (col 64 = ones) ----
        def emit_v_st(st):
            if st % 4 == 0:
                if "pre" in vstate:
                    xv_t = vstate.pop("pre")
                else:
                    xv_t = p_xs.tile([128, DMT, 512], dmm, tag="xs",
                                     name=f"xv{st}")
                    nc.sync.dma_start(out=xv_t[:], in_=xv[st // 4])
                vstate["xv"] = xv_t
            xv_t = vstate["xv"]
            sub = (st % 4) * 128
            ps = p_ps.tile([128, 1024], f32, tag="ps", name=f"psv{st}")
            for a in range(DMT):
                nc.tensor.matmul(
                    out=ps[:, 0:GD],
                    lhsT=xv_t[:, a, sub:sub + 128],
                    rhs=wv_sb[:, a, :],
                    start=(a == 0), stop=(a == DMT - 1),
                )
            vt = p_v.tile([128, GH, 65], dmm, tag="v", name=f"v{st}")
            nc.vector.tensor_copy(
                out=vt[:, :, 0:DK],
                in_=ps[:, 0:GD].rearrange("p (h d) -> p h d", h=GH),
            )
            nc.vector.tensor_copy(
                out=vt[:, :, DK:65],
                in_=ones_sb.unsqueeze(1).broadcast_to([128, GH, 1]))
            v_sb.append(vt)

        def v_filler(st):
            def emit():
                emit_v_st(st)
            return emit

        # ---- Q/K projections for head pair c; one group = one s-chunk ----
        def alloc_pair(c):
            for hh in range(2):
                hg = 2 * c + hh
                qt_sb[hg] = p_qt.tile([128, S], dmm, tag="qt", name=f"qt{hg}")
                kt_sb[hg] = p_kt.tile([128, S], dmm, tag="kt", name=f"kt{hg}")

        def proj_chunks(c, which, sc, xs_pre=None):
            """Q^T/K^T slice for heads 2c,2c+1 over s-range sc, split into
            two 4-MM chunks so filler work interleaves with the attention
            matmul stream without starving ACT. Head tile [128, S] holds
            its 64 dims twice (rows 0-63 and 64-127) so consecutive kt
            score matmuls alternate PE row groups and run concurrently."""
            src, wsb, bsb = ((xq, wq_sb, bq_sb) if which == "q"
                            else (xk, wk_sb, bk_sb))
            dsts = ([qt_sb[2 * c], qt_sb[2 * c + 1]] if which == "q"
                    else [kt_sb[2 * c], kt_sb[2 * c + 1]])
            state = {}

            def emit_a():
                if xs_pre is not None:
                    xs = xs_pre
                else:
                    xs = p_xs.tile([128, DMT, SC], dmm, tag="xs",
                                   name=f"xs{which}{c}_{sc}")
                    nc.sync.dma_start(out=xs[:], in_=src[sc])
                ps = p_ps.tile([128, 1024], f32, tag="ps",
                               name=f"psp{which}{c}_{sc}")
                for a in range(4):
                    nc.tensor.matmul(
                        out=ps[:, 0:SC],
                        lhsT=wsb[:, a, c * 128:(c + 1) * 128],
                        rhs=xs[:, a, :],
                        start=(a == 0), stop=False,
                    )
                state["xs"], state["ps"] = xs, ps

            def emit_b():
                xs, ps = state["xs"], state["ps"]
                for a in range(4, DMT):
                    nc.tensor.matmul(
                        out=ps[:, 0:SC],
                        lhsT=wsb[:, a, c * 128:(c + 1) * 128],
                        rhs=xs[:, a, :],
                        start=False, stop=(a == DMT - 1),
                    )
                s0, s1 = sc * SC, (sc + 1) * SC
                # head 2c native rows 0-63; head 2c+1 native rows 64-127
                nc.vector.tensor_scalar_add(
                    out=dsts[0][0:DK, s0:s1], in0=ps[0:DK, 0:SC],
                    scalar1=bsb[0:DK, c:c + 1])
                nc.vector.tensor_scalar_add(
                    out=dsts[1][DK:128, s0:s1], in0=ps[DK:128, 0:SC],
                    scalar1=bsb[DK:128, c:c + 1])
                # duplicate this slice into the other half right away
                # (SBUF->SBUF DMA) so scores kt for this s-range unblock
                nc.sync.dma_start(out=dsts[0][DK:128, s0:s1],
                                  in_=dsts[0][0:DK, s0:s1])
                nc.sync.dma_start(out=dsts[1][0:DK, s0:s1],
                                  in_=dsts[1][DK:128, s0:s1])
            return [emit_a, emit_b]

        # ---- output projection for one 128-row q tile, two 4-MM chunks ----
        def fin_chunks(qt_i):
            state = {}

            def emit_a():
                ps = p_ps.tile([128, 1024], f32, tag="ps", name=f"pso{qt_i}")
                for cc in range(2):
                    for half in range(2):
                        nc.tensor.matmul(
                            out=ps[:, half * 512:(half + 1) * 512],
                            lhsT=ot_sb[cc][:, qt_i * 128:(qt_i + 1) * 128],
                            rhs=wo_sb[:, cc, half * 512:(half + 1) * 512],
                            start=(cc == 0), stop=False,
                        )
                state["ps"] = ps

            def emit_b():
                ps = state["ps"]
                for cc in range(2, NCH):
                    for half in range(2):
                        nc.tensor.matmul(
                            out=ps[:, half * 512:(half + 1) * 512],
                            lhsT=ot_sb[cc][:, qt_i * 128:(qt_i + 1) * 128],
                            rhs=wo_sb[:, cc, half * 512:(half + 1) * 512],
                            start=False, stop=(cc == NCH - 1),
                        )
                st = p_st.tile([128, D], f32, tag="st", name=f"st{qt_i}")
                nc.vector.tensor_copy(out=st[:], in_=ps[:])
                nc.sync.dma_start(out=out[qt_i * 128:(qt_i + 1) * 128, :],
                                  in_=st[:])
            return [emit_a, emit_b]

        # ---- one attention loop: head hg = 2c+hh, q columns qc*QC.. ----
        def attention_loop(c, qc, hh, slots=(), fillers=None, queue=None,
                           qcap=10, qw=QC):
            """fillers: per-loop whole-group callbacks popped at `slots`
            (two per 2-kt block); queue: global 4-MM chunk queue popped
            once per block with kb <= qcap."""
            slots = set(slots)
            fillers = fillers if fillers is not None else []
            hg = 2 * c + hh
            if ot_sb[c] is None:
                ot_sb[c] = p_ot.tile([128, S], dmm, tag="ot", name=f"ot{c}")
            qb = qc * 512
            pv_ps = p_pv.tile([65, qw], f32, tag="pv",
                              name=f"pv{c}_{qc}_{hh}")
            pts = {}

            def emit_pv(kt_i):
                pt = pts.pop(kt_i)
                for half in range(qw // 512):
                    nc.tensor.matmul(
                        out=pv_ps[:, half * 512:(half + 1) * 512],
                        lhsT=v_sb[kt_i][:, hg, :],
                        rhs=pt[:, half * 512:(half + 1) * 512],
                        start=(kt_i == 0), stop=(kt_i == KT - 1),
                    )

            # kt processed in blocks of 2: both kt's score halves are
            # interleaved (alternating 64-row PE groups -> the two streams
            # run concurrently), then the PVs of the PREVIOUS block, then
            # filler chunks. PV lags its exp by one block so the in-order
            # PE stream never parks on an exp wait, and same-weight-shape
            # matmuls stay contiguous (fewer exposed LDWEIGHTS).
            for kb in range(0, KT, 2):
                pss = []
                for j in range(2):
                    pss.append(p_ps.tile([128, qw], f32, tag="ps",
                                         name=f"pss{c}_{qc}_{kb + j}_{hh}"))
                for half in range(qw // 512):
                    q0 = qb + half * 512
                    for j in range(2):
                        kt_i = kb + j
                        rg = DK * j
                        nc.tensor.matmul(
                            out=pss[j][:, half * 512:(half + 1) * 512],
                            lhsT=kt_sb[hg][rg:rg + DK,
                                           kt_i * 128:(kt_i + 1) * 128],
                            rhs=qt_sb[hg][rg:rg + DK, q0:q0 + 512],
                            start=True, stop=True,
                        )
                for j in range(2):
                    kt_i = kb + j
                    pt = p_pt.tile([128, qw], dmm, tag="pt",
                                   name=f"pt{c}_{qc}_{kt_i}_{hh}")
                    nc.scalar.activation(pt[:], pss[j][:], Exp,
                                         bias=0.0, scale=0.125)
                    pts[kt_i] = pt
                if kb >= 2:
                    emit_pv(kb - 2)
                    emit_pv(kb - 1)
                if fillers:
                    if kb in slots:
                        fillers.pop(0)()
                    if kb + 1 in slots and fillers:
                        fillers.pop(0)()
                elif queue and kb <= qcap:
                    queue.pop(0)()
            emit_pv(KT - 2)
            emit_pv(KT - 1)
            # evict PV psum right away to release its bank pair
            ovt = p_ov.tile([65, qw], f32, tag="ov", name=f"ov{c}_{qc}_{hh}")
            nc.vector.tensor_copy(out=ovt[:], in_=pv_ps[:])
            # normalize off the critical path:
            # O^T = PV[0:64] * broadcast(1 / PV[64])
            zs = p_zr.tile([DK, qw // DK], f32, tag="zs",
                           name=f"zs{c}_{qc}_{hh}")
            nc.sync.dma_start(out=zs[:], in_=ovt[DK:DK + 1, :])
            nc.vector.reciprocal(out=zs[:], in_=zs[:])
            zr = p_zr.tile([1, qw], f32, tag="zr", name=f"zr{c}_{qc}_{hh}")
            nc.sync.dma_start(out=zr[:], in_=zs[:])
            rb = p_rb.tile([DK, qw], f32, tag="rb", name=f"rb{c}_{qc}_{hh}")
            nc.gpsimd.partition_broadcast(rb[:], zr[:], channels=DK)
            if hh == 0:
                nc.vector.tensor_mul(
                    out=ot_sb[c][0:DK, qb:qb + qw],
                    in0=ovt[0:DK, :], in1=rb[:])
            else:
                tmp = p_rb.tile([DK, qw], dmm, tag="rb", name=f"tmp{c}_{qc}")
                nc.vector.tensor_mul(out=tmp[:], in0=ovt[0:DK, :], in1=rb[:])
                nc.sync.dma_start(
                    out=ot_sb[c][DK:128, qb:qb + qw],
                    in_=tmp[:])

        # ================= emission =================
        # prologue: pair-0 Q/K s-chunks 0/1 + V chunk 0 (inputs pre-issued
        # in the DMA waves above)
        alloc_pair(0)
        for e in proj_chunks(0, "k", 0, xs_pre=xs_k0):
            e()
        for e in proj_chunks(0, "q", 0, xs_pre=xs_q0):
            e()
        for e in proj_chunks(0, "q", 1, xs_pre=xs_q1):
            e()

        alloc_pair(1)
        alloc_pair(2)
        alloc_pair(3)

        # loop (0,0,0) consumes v0..v15 + k1..k3 just-in-time as whole
        # groups (it is PE-bound regardless); everything downstream drains
        # from the global 4-MM chunk queue, one chunk per odd kt
        k01 = proj_chunks(0, "k", 1)
        k02 = proj_chunks(0, "k", 2)
        k03 = proj_chunks(0, "k", 3)
        f000 = [v_filler(0),
                lambda: (v_filler(1)(), k01[0]()),
                lambda: (v_filler(2)(), k01[1]()),
                lambda: (v_filler(3)(), k02[0]()),
                lambda: (v_filler(4)(), k02[1]()),
                lambda: (v_filler(5)(), k03[0]()),
                lambda: (v_filler(6)(), k03[1]()),
                v_filler(7), v_filler(8), v_filler(9),
                v_filler(10), v_filler(11), v_filler(12), v_filler(13),
                v_filler(14), v_filler(15)]

        FQ = []
        for cc, w, sc in [(0, "q", 2), (0, "q", 3),
                          (1, "k", 0), (1, "q", 0), (1, "q", 1), (1, "k", 1),
                          (1, "k", 2), (1, "q", 2), (1, "k", 3), (1, "q", 3),
                          (2, "k", 0), (2, "q", 0), (2, "q", 1), (2, "k", 1),
                          (2, "k", 2), (2, "q", 2), (2, "k", 3), (2, "q", 3),
                          (3, "k", 0), (3, "q", 0), (3, "q", 1), (3, "k", 1),
                          (3, "k", 2), (3, "q", 2), (3, "k", 3), (3, "q", 3)]:
            FQ.extend(proj_chunks(cc, w, sc))

        FIN = []
        for qt_i in range(12):
            FIN.extend(fin_chunks(qt_i))

        for c in range(NCH):
            for qc in range(NQC):
                for hh in range(2):
                    if (c, qc, hh) == (0, 0, 0):
                        attention_loop(c, 0, 0, slots=range(0, 16),
                                       fillers=f000)
                    elif c == 3 and qc == 1:
                        pass  # emitted below in tail-friendly order
                    else:
                        attention_loop(c, qc * 2, hh, queue=FQ, qcap=10)
        # pair-3 qc1: head 1 first (full width), then head 0 as two
        # 512-wide half loops -> q tiles 8-11 finish one half-loop early
        # and their output projection overlaps the last attention work;
        # only q tiles 12-15 remain after all attention
        attention_loop(3, 2, 1, queue=FIN, qcap=14)
        attention_loop(3, 2, 0, queue=FIN, qcap=14, qw=512)
        attention_loop(3, 3, 0, queue=FIN, qcap=14, qw=512)
        for e in FIN:
            e()
        for qt_i in range(12, KT):
            a, b2 = fin_chunks(qt_i)
            a()
            b2()

    nc.compile()
    return nc


def get_program():
    if "nc" not in _CACHE:
        _CACHE["nc"] = _build_program()
    return _CACHE["nc"]


def make_in_maps(inputs):
    dt = _np_mm_dtype()
    q = np.asarray(inputs["query"], np.float32)
    k = np.asarray(inputs["key"], np.float32)
    v = np.asarray(inputs["value"], np.float32)
    Wq = np.asarray(inputs["Wq"], np.float32)
    Wk = np.asarray(inputs["Wk"], np.float32)
    Wv = np.asarray(inputs["Wv"], np.float32)
    Wo = np.asarray(inputs["Wo"], np.float32)
    bq = np.asarray(inputs["bq"], np.float32)
    bk = np.asarray(inputs["bk"], np.float32)

    def slab(x):
        # [S, D] -> x.T [D, S] -> [sc, p, a, s] contiguous slabs
        return np.ascontiguousarray(
            x.T.reshape(DMT, 128, NSC, SC).transpose(2, 1, 0, 3))

    def wtile(W):
        # [D, GD_slice] -> [p, a, d]
        return np.ascontiguousarray(W.reshape(DMT, 128, -1).transpose(1, 0, 2))

    in_maps = []
    for core in range(NCORES):
        b, g = core // 2, core % 2
        sl = slice(g * GD, (g + 1) * GD)
        in_maps.append({
            "xq": slab(q[b]).astype(dt),
            "xk": slab(k[b]).astype(dt),
            "xv": slab(v[b]).astype(dt),
            "wq": wtile(Wq[:, sl]).astype(dt),
            "wk": wtile(Wk[:, sl]).astype(dt),
            "wv": wtile(Wv[:, sl]).astype(dt),
            "wo": np.ascontiguousarray(
                Wo[sl, :].reshape(NCH, 128, D).transpose(1, 0, 2)).astype(dt),
            "bq": np.ascontiguousarray(bq[sl].reshape(NCH, 128).T),
            "bk": np.ascontiguousarray(bk[sl].reshape(NCH, 128).T),
        })
    return in_maps


def combine_outputs(results, inputs):
    Wo = np.asarray(inputs["Wo"], np.float32)
    bv = np.asarray(inputs["bv"], np.float32)
    bo = np.asarray(inputs["bo"], np.float32)
    out = np.empty((B, S, D), np.float32)
    for b in range(B):
        out[b] = results[2 * b]["out"] + results[2 * b + 1]["out"]
    out += bv @ Wo + bo
    return out


def kernel(**inputs):
    from concourse.bass_utils import run_bass_kernel_spmd
    nc = get_program()
    in_maps = make_in_maps(inputs)
    res = run_bass_kernel_spmd(nc, in_maps, list(range(NCORES)))
    return combine_outputs(res.results, inputs)


# revision 32
# speedup vs baseline: 1.1695x; 1.1695x over previous
"""Multi-head attention (B=4, S=2048, D=1024, H=16) on 8 trn2 NeuronCores.

Sharding: core = (batch b, head-group g) with b = core//2, g = core%2.
Each core handles one batch and 8 heads (512 of the 1024 d_model dims):
  - host pre-tiles query/key/value[b] and the weight slices into the exact
    SBUF layouts ([sc, 128, a, s] slabs / [128, a, d] weights) so every DMA
    is a contiguous 8KB-per-partition transfer (no strided descriptors)
  - device computes Q^T, K^T (head dims on partitions) and V (natural),
    attention with *transposed* scores S^T = K_h @ Q_h^T so softmax's
    denominator comes out of the PV matmul via a ones-column appended to V
  - output projection vs Wo[g*512:(g+1)*512, :] gives a partial [2048,1024]
  - host sums the two group partials per batch and adds bv@Wo + bo
Matmul operand dtype is MM_DT (bf16 default). PSUM accumulation and the
softmax normalization chain stay fp32.

Scheduling (the exp stream on ScalarE, 256x [128,1024] ACTIVATEs at
~1.15us each, is the pipeline's pace-setter; PE stream work is ~85-95%
of it):
  - kt is processed in blocks of 2: both kt's score matmul halves are
    interleaved with alternating 64-row PE groups so the two streams run
    concurrently on the systolic array; the previous block's PV matmuls
    follow (PV lags its exp by one block so the in-order PE queue never
    parks on an exp wait); then one 4-MM projection chunk from a global
    filler queue (whole 8-MM groups starved the exp stream).
  - prologue DMAs go in two dependency-gated waves (the DMA engines
    fair-share HBM bandwidth, so ungated transfers all land at the same
    late instant): wave1 = everything the first scores block + first PV
    needs, wave2 = Wo.
  - pair 3's last q range runs head 1 first, then head 0 as two 512-wide
    half-loops, so q tiles 8-11 project during the final attention work
    and only q tiles 12-15 remain after it.
"""

import os
import numpy as np
from contextlib import ExitStack

B = 4
S = 2048
D = 1024
H = 16
DK = 64
NCORES = 8
GH = 8          # heads per core (group)
GD = GH * DK    # 512 head dims per core
NCH = GD // 128  # 4 chunks of 128 output dims
KT = S // 128    # 16 key tiles
QC = 1024        # q chunk width for attention
NQC = S // QC    # 2
SC = 512         # s chunk width for projections
NSC = S // SC    # 4
DMT = D // 128   # 8 d_model tiles

MM_DT = os.environ.get("MM_DT", "bf16")  # "bf16" | "f32r"

_CACHE = {}


def _np_mm_dtype():
    if MM_DT == "bf16":
        import ml_dtypes
        return ml_dtypes.bfloat16
    return np.float32


def _build_program():
    import concourse.mybir as mybir
    import concourse.tile as tile
    from concourse import bacc
    from concourse.tile_rust import add_dep_helper

    f32 = mybir.dt.float32
    dmm = mybir.dt.bfloat16 if MM_DT == "bf16" else mybir.dt.float32r

    nc = bacc.Bacc("TRN2", target_bir_lowering=False, debug=False,
                   num_devices=NCORES)

    xq = nc.dram_tensor("xq", [NSC, 128, DMT, SC], dmm,
                        kind="ExternalInput").ap()
    xk = nc.dram_tensor("xk", [NSC, 128, DMT, SC], dmm,
                        kind="ExternalInput").ap()
    xv = nc.dram_tensor("xv", [NSC, 128, DMT, SC], dmm,
                        kind="ExternalInput").ap()
    wq = nc.dram_tensor("wq", [128, DMT, GD], dmm, kind="ExternalInput").ap()
    wk = nc.dram_tensor("wk", [128, DMT, GD], dmm, kind="ExternalInput").ap()
    wv = nc.dram_tensor("wv", [128, DMT, GD], dmm, kind="ExternalInput").ap()
    wo = nc.dram_tensor("wo", [128, NCH, D], dmm, kind="ExternalInput").ap()
    bq = nc.dram_tensor("bq", [128, NCH], f32, kind="ExternalInput").ap()
    bk = nc.dram_tensor("bk", [128, NCH], f32, kind="ExternalInput").ap()
    out = nc.dram_tensor("out", [S, D], f32, kind="ExternalOutput").ap()

    Exp = mybir.ActivationFunctionType.Exp

    with tile.TileContext(nc) as tc, ExitStack() as ctx:
        # ---- pools (slots are statically reserved per tag) ----
        p_qt = ctx.enter_context(tc.tile_pool(name="qt", bufs=GH))
        p_kt = ctx.enter_context(tc.tile_pool(name="kt", bufs=GH))
        p_v = ctx.enter_context(tc.tile_pool(name="v", bufs=KT))
        p_ot = ctx.enter_context(tc.tile_pool(name="ot", bufs=NCH))
        p_wvo = ctx.enter_context(tc.tile_pool(name="wvo", bufs=1))
        p_wc = ctx.enter_context(tc.tile_pool(name="wc", bufs=1))
        p_bias = ctx.enter_context(tc.tile_pool(name="bias", bufs=1))
        p_xs = ctx.enter_context(tc.tile_pool(name="xs", bufs=4))
        p_pt = ctx.enter_context(tc.tile_pool(name="pt", bufs=7))
        p_zr = ctx.enter_context(tc.tile_pool(name="zr", bufs=2))
        p_rb = ctx.enter_context(tc.tile_pool(name="rb", bufs=2))
        p_st = ctx.enter_context(tc.tile_pool(name="st", bufs=2))
        p_ov = ctx.enter_context(tc.tile_pool(name="ov", bufs=2))
        # PSUM: ps 3 slots x 2 banks + pv 1 slot x 2 banks = all 8 banks
        p_ps = ctx.enter_context(tc.tile_pool(name="ps", bufs=3, space="PSUM"))
        p_pv = ctx.enter_context(tc.tile_pool(name="pv", bufs=1, space="PSUM"))

        # ---- biases + ones (tiny, sync queue) ----
        bq_sb = p_bias.tile([128, NCH], f32, tag="bq")
        nc.sync.dma_start(out=bq_sb[:], in_=bq)
        bk_sb = p_bias.tile([128, NCH], f32, tag="bk")
        nc.sync.dma_start(out=bk_sb[:], in_=bk)
        ones_sb = p_bias.tile([128, 1], f32, tag="ones")
        nc.vector.memset(ones_sb[:], 1.0)

        # ---- prologue DMAs in two dependency-gated waves: the DMA engines
        # fair-share HBM bandwidth across everything in flight, so gate the
        # non-critical wave behind the 5MB the first scores matmul needs
        # (wk, wq, xs k0/q0/q1). V arrives in wave2; the exp stream runs
        # ~7 kt ahead of PV, which absorbs the later V start.
        wk_sb = p_wc.tile([128, DMT, GD], dmm, tag="wkc", name="wk_sb")
        i_wk = nc.gpsimd.dma_start(out=wk_sb[:], in_=wk)
        wq_sb = p_wc.tile([128, DMT, GD], dmm, tag="wqc", name="wq_sb")
        i_wq = nc.gpsimd.dma_start(out=wq_sb[:], in_=wq)
        xs_k0 = p_xs.tile([128, DMT, SC], dmm, tag="xs", name="xs_k0")
        i_k0 = nc.sync.dma_start(out=xs_k0[:], in_=xk[0])
        xs_q0 = p_xs.tile([128, DMT, SC], dmm, tag="xs", name="xs_q0")
        i_q0 = nc.sync.dma_start(out=xs_q0[:], in_=xq[0])
        xs_q1 = p_xs.tile([128, DMT, SC], dmm, tag="xs", name="xs_q1")
        i_q1 = nc.scalar.dma_start(out=xs_q1[:], in_=xq[1])
        xv_0 = p_xs.tile([128, DMT, 512], dmm, tag="xs", name="xv0")
        i_xv0 = nc.scalar.dma_start(out=xv_0[:], in_=xv[0])
        vstate = {"pre": xv_0}
        wv_sb = p_wvo.tile([128, DMT, GD], dmm, tag="wvo", name="wv_sb")
        i_wv = nc.gpsimd.dma_start(out=wv_sb[:], in_=wv)
        wave1 = [i_wk, i_wq, i_k0, i_q0, i_q1, i_xv0, i_wv]

        wo_sb = p_wvo.tile([128, NCH, D], dmm, tag="wo", name="wo_sb")
        i_wo = nc.scalar.dma_start(out=wo_sb[:], in_=wo)
        wave2 = [i_wo]

        for later in wave2:
            for earlier in wave1:
                add_dep_helper(later.ins, earlier.ins,
                               reason="dma wave order")

        qt_sb = [None] * GH
        kt_sb = [None] * GH
        ot_sb = [None] * NCH
        v_sb = []

        # ---- V projection: V_sb[st] = [128 s, GH, 65] (col 64 = ones) ----
        def emit_v_st(st):
            if st % 4 == 0:
                if "pre" in vstate:
                    xv_t = vstate.pop("pre")
                else:
                    xv_t = p_xs.tile([128, DMT, 512], dmm, tag="xs",
                                     name=f"xv{st}")
                    nc.sync.dma_start(out=xv_t[:], in_=xv[st // 4])
                vstate["xv"] = xv_t
            xv_t = vstate["xv"]
            sub = (st % 4) * 128
            ps = p_ps.tile([128, 1024], f32, tag="ps", name=f"psv{st}")
            for a in range(DMT):
                nc.tensor.matmul(
                    out=ps[:, 0:GD],
                    lhsT=xv_t[:, a, sub:sub + 128],
                    rhs=wv_sb[:, a, :],
                    start=(a == 0), stop=(a == DMT - 1),
                )
            vt = p_v.tile([128, GH, 65], dmm, tag="v", name=f"v{st}")
            nc.vector.tensor_copy(
                out=vt[:, :, 0:DK],
                in_=ps[:, 0:GD].rearrange("p (h d) -> p h d", h=GH),
            )
            nc.vector.tensor_copy(
                out=vt[:, :, DK:65],
                in_=ones_sb.unsqueeze(1).broadcast_to([128, GH, 1]))
            v_sb.append(vt)

        def v_filler(st):
            def emit():
                emit_v_st(st)
            return emit

        # ---- Q/K projections for head pair c; one group = one s-chunk ----
        def alloc_pair(c):
            for hh in range(2):
                hg = 2 * c + hh
                qt_sb[hg] = p_qt.tile([128, S], dmm, tag="qt", name=f"qt{hg}")
                kt_sb[hg] = p_kt.tile([128, S], dmm, tag="kt", name=f"kt{hg}")

        def proj_chunks(c, which, sc, xs_pre=None):
            """Q^T/K^T slice for heads 2c,2c+1 over s-range sc, split into
            two 4-MM chunks so filler work interleaves with the attention
            matmul stream without starving ACT. Head tile [128, S] holds
            its 64 dims twice (rows 0-63 and 64-127) so consecutive kt
            score matmuls alternate PE row groups and run concurrently."""
            src, wsb, bsb = ((xq, wq_sb, bq_sb) if which == "q"
                            else (xk, wk_sb, bk_sb))
            dsts = ([qt_sb[2 * c], qt_sb[2 * c + 1]] if which == "q"
                    else [kt_sb[2 * c], kt_sb[2 * c + 1]])
            state = {}

            def emit_a():
                if xs_pre is not None:
                    xs = xs_pre
                else:
                    xs = p_xs.tile([128, DMT, SC], dmm, tag="xs",
                                   name=f"xs{which}{c}_{sc}")
                    nc.sync.dma_start(out=xs[:], in_=src[sc])
                ps = p_ps.tile([128, 1024], f32, tag="ps",
                               name=f"psp{which}{c}_{sc}")
                for a in range(4):
                    nc.tensor.matmul(
                        out=ps[:, 0:SC],
                        lhsT=wsb[:, a, c * 128:(c + 1) * 128],
                        rhs=xs[:, a, :],
                        start=(a == 0), stop=False,
                    )
                state["xs"], state["ps"] = xs, ps

            def emit_b():
                xs, ps = state["xs"], state["ps"]
                for a in range(4, DMT):
                    nc.tensor.matmul(
                        out=ps[:, 0:SC],
                        lhsT=wsb[:, a, c * 128:(c + 1) * 128],
                        rhs=xs[:, a, :],
                        start=False, stop=(a == DMT - 1),
                    )
                s0, s1 = sc * SC, (sc + 1) * SC
                # head 2c native rows 0-63; head 2c+1 native rows 64-127
                nc.vector.tensor_scalar_add(
                    out=dsts[0][0:DK, s0:s1], in0=ps[0:DK, 0:SC],
                    scalar1=bsb[0:DK, c:c + 1])
                nc.vector.tensor_scalar_add(
                    out=dsts[1][DK:128, s0:s1], in0=ps[DK:128, 0:SC],
                    scalar1=bsb[DK:128, c:c + 1])
                # duplicate this slice into the other half right away
                # (SBUF->SBUF DMA) so scores kt for this s-range unblock
                nc.sync.dma_start(out=dsts[0][DK:128, s0:s1],
                                  in_=dsts[0][0:DK, s0:s1])
                nc.sync.dma_start(out=dsts[1][0:DK, s0:s1],
                                  in_=dsts[1][DK:128, s0:s1])
            return [emit_a, emit_b]

        # ---- output projection for one 128-row q tile, two 4-MM chunks ----
        def fin_chunks(qt_i):
            state = {}

            def emit_a():
                ps = p_ps.tile([128, 1024], f32, tag="ps", name=f"pso{qt_i}")
                for cc in range(2):
                    for half in range(2):
                        nc.tensor.matmul(
                            out=ps[:, half * 512:(half + 1) * 512],
                            lhsT=ot_sb[cc][:, qt_i * 128:(qt_i + 1) * 128],
                            rhs=wo_sb[:, cc, half * 512:(half + 1) * 512],
                            start=(cc == 0), stop=False,
                        )
                state["ps"] = ps

            def emit_b():
                ps = state["ps"]
                for cc in range(2, NCH):
                    for half in range(2):
                        nc.tensor.matmul(
                            out=ps[:, half * 512:(half + 1) * 512],
                            lhsT=ot_sb[cc][:, qt_i * 128:(qt_i + 1) * 128],
                            rhs=wo_sb[:, cc, half * 512:(half + 1) * 512],
                            start=False, stop=(cc == NCH - 1),
                        )
                st = p_st.tile([128, D], f32, tag="st", name=f"st{qt_i}")
                nc.vector.tensor_copy(out=st[:], in_=ps[:])
                nc.sync.dma_start(out=out[qt_i * 128:(qt_i + 1) * 128, :],
                                  in_=st[:])
            return [emit_a, emit_b]

        # ---- one attention loop: head hg = 2c+hh, q columns qc*QC.. ----
        def attention_loop(c, qc, hh, slots=(), fillers=None, queue=None,
                           qcap=10, qw=QC):
            """fillers: per-loop whole-group callbacks popped at `slots`
            (two per 2-kt block); queue: global 4-MM chunk queue popped
            once per block with kb <= qcap."""
            slots = set(slots)
            fillers = fillers if fillers is not None else []
            hg = 2 * c + hh
            if ot_sb[c] is None:
                ot_sb[c] = p_ot.tile([128, S], dmm, tag="ot", name=f"ot{c}")
            qb = qc * 512
            pv_ps = p_pv.tile([65, qw], f32, tag="pv",
                              name=f"pv{c}_{qc}_{hh}")
            pts = {}

            def emit_pv(kt_i):
                pt = pts.pop(kt_i)
                for half in range(qw // 512):
                    nc.tensor.matmul(
                        out=pv_ps[:, half * 512:(half + 1) * 512],
                        lhsT=v_sb[kt_i][:, hg, :],
                        rhs=pt[:, half * 512:(half + 1) * 512],
                        start=(kt_i == 0), stop=(kt_i == KT - 1),
                    )

            # kt processed in blocks of 2: both kt's score halves are
            # interleaved (alternating 64-row PE groups -> the two streams
            # run concurrently), then the PVs of the PREVIOUS block, then
            # filler chunks. PV lags its exp by one block so the in-order
            # PE stream never parks on an exp wait, and same-weight-shape
            # matmuls stay contiguous (fewer exposed LDWEIGHTS).
            for kb in range(0, KT, 2):
                pss = []
                for j in range(2):
                    pss.append(p_ps.tile([128, qw], f32, tag="ps",
                                         name=f"pss{c}_{qc}_{kb + j}_{hh}"))
                for half in range(qw // 512):
                    q0 = qb + half * 512
                    for j in range(2):
                        kt_i = kb + j
                        rg = DK * j
                        nc.tensor.matmul(
                            out=pss[j][:, half * 512:(half + 1) * 512],
                            lhsT=kt_sb[hg][rg:rg + DK,
                                           kt_i * 128:(kt_i + 1) * 128],
                            rhs=qt_sb[hg][rg:rg + DK, q0:q0 + 512],
                            start=True, stop=True,
                        )
                for j in range(2):
                    kt_i = kb + j
                    pt = p_pt.tile([128, qw], dmm, tag="pt",
                                   name=f"pt{c}_{qc}_{kt_i}_{hh}")
                    nc.scalar.activation(pt[:], pss[j][:], Exp,
                                         bias=0.0, scale=0.125)
                    pts[kt_i] = pt
                if kb >= 2:
                    emit_pv(kb - 2)
                    emit_pv(kb - 1)
                if fillers:
                    if kb in slots:
                        fillers.pop(0)()
                    if kb + 1 in slots and fillers:
                        fillers.pop(0)()
                elif queue and kb <= qcap:
                    queue.pop(0)()
            emit_pv(KT - 2)
            emit_pv(KT - 1)
            # evict PV psum right away to release its bank pair
            ovt = p_ov.tile([65, qw], f32, tag="ov", name=f"ov{c}_{qc}_{hh}")
            nc.vector.tensor_copy(out=ovt[:], in_=pv_ps[:])
            # normalize off the critical path:
            # O^T = PV[0:64] * broadcast(1 / PV[64])
            zs = p_zr.tile([DK, qw // DK], f32, tag="zs",
                           name=f"zs{c}_{qc}_{hh}")
            nc.sync.dma_start(out=zs[:], in_=ovt[DK:DK + 1, :])
            nc.vector.reciprocal(out=zs[:], in_=zs[:])
            zr = p_zr.tile([1, qw], f32, tag="zr", name=f"zr{c}_{qc}_{hh}")
            nc.sync.dma_start(out=zr[:], in_=zs[:])
            rb = p_rb.tile([DK, qw], f32, tag="rb", name=f"rb{c}_{qc}_{hh}")
            nc.gpsimd.partition_broadcast(rb[:], zr[:], channels=DK)
            if hh == 0:
                nc.vector.tensor_mul(
                    out=ot_sb[c][0:DK, qb:qb + qw],
                    in0=ovt[0:DK, :], in1=rb[:])
            else:
                tmp = p_rb.tile([DK, qw], dmm, tag="rb", name=f"tmp{c}_{qc}")
                nc.vector.tensor_mul(out=tmp[:], in0=ovt[0:DK, :], in1=rb[:])
                nc.sync.dma_start(
                    out=ot_sb[c][DK:128, qb:qb + qw],
                    in_=tmp[:])

        # ================= emission =================
        # prologue: pair-0 Q/K s-chunks 0/1 + V chunk 0 (inputs pre-issued
        # in the DMA waves above)
        alloc_pair(0)
        for e in proj_chunks(0, "k", 0, xs_pre=xs_k0):
            e()
        for e in proj_chunks(0, "q", 0, xs_pre=xs_q0):
            e()
        for e in proj_chunks(0, "q", 1, xs_pre=xs_q1):
            e()

        alloc_pair(1)
        alloc_pair(2)
        alloc_pair(3)

        # loop (0,0,0) consumes v0..v15 + k1..k3 just-in-time as whole
        # groups (it is PE-bound regardless); everything downstream drains
        # from the global 4-MM chunk queue, one chunk per odd kt
        k01 = proj_chunks(0, "k", 1)
        k02 = proj_chunks(0, "k", 2)
        k03 = proj_chunks(0, "k", 3)
        f000 = [v_filler(0),
                lambda: (v_filler(1)(), k01[0](), k01[1]()),
                v_filler(2),
                lambda: (v_filler(3)(), k02[0](), k02[1]()),
                v_filler(4),
                lambda: (v_filler(5)(), k03[0](), k03[1]()),
                v_filler(6), v_filler(7), v_filler(8), v_filler(9),
                v_filler(10), v_filler(11), v_filler(12), v_filler(13),
                v_filler(14), v_filler(15)]

        FQ = []
        for cc, w, sc in [(0, "q", 2), (0, "q", 3),
                          (1, "k", 0), (1, "q", 0), (1, "q", 1), (1, "k", 1),
                          (1, "k", 2), (1, "q", 2), (1, "k", 3), (1, "q", 3),
                          (2, "k", 0), (2, "q", 0), (2, "q", 1), (2, "k", 1),
                          (2, "k", 2), (2, "q", 2), (2, "k", 3), (2, "q", 3),
                          (3, "k", 0), (3, "q", 0), (3, "q", 1), (3, "k", 1),
                          (3, "k", 2), (3, "q", 2), (3, "k", 3), (3, "q", 3)]:
            FQ.extend(proj_chunks(cc, w, sc))

        FIN = []
        for qt_i in range(12):
            FIN.extend(fin_chunks(qt_i))

        for c in range(NCH):
            for qc in range(NQC):
                for hh in range(2):
                    if (c, qc, hh) == (0, 0, 0):
                        attention_loop(c, 0, 0, slots=range(0, 16),
                                       fillers=f000)
                    elif c == 3 and qc == 1:
                        pass  # emitted below in tail-friendly order
                    else:
                        attention_loop(c, qc * 2, hh, queue=FQ, qcap=10)
        # pair-3 qc1: head 1 first (full width), then head 0 as two
        # 512-wide half loops -> q tiles 8-11 finish one half-loop early
        # and their output projection overlaps the last attention work;
        # only q tiles 12-15 remain after all attention
        attention_loop(3, 2, 1, queue=FIN, qcap=14)
        attention_loop(3, 2, 0, queue=FIN, qcap=14, qw=512)
        attention_loop(3, 3, 0, queue=FIN, qcap=14, qw=512)
        for e in FIN:
            e()
        for qt_i in range(12, KT):
            a, b2 = fin_chunks(qt_i)
            a()
            b2()

    nc.compile()
    return nc


def get_program():
    if "nc" not in _CACHE:
        _CACHE["nc"] = _build_program()
    return _CACHE["nc"]


def make_in_maps(inputs):
    dt = _np_mm_dtype()
    q = np.asarray(inputs["query"], np.float32)
    k = np.asarray(inputs["key"], np.float32)
    v = np.asarray(inputs["value"], np.float32)
    Wq = np.asarray(inputs["Wq"], np.float32)
    Wk = np.asarray(inputs["Wk"], np.float32)
    Wv = np.asarray(inputs["Wv"], np.float32)
    Wo = np.asarray(inputs["Wo"], np.float32)
    bq = np.asarray(inputs["bq"], np.float32)
    bk = np.asarray(inputs["bk"], np.float32)

    def slab(x):
        # [S, D] -> x.T [D, S] -> [sc, p, a, s] contiguous slabs
        return np.ascontiguousarray(
            x.T.reshape(DMT, 128, NSC, SC).transpose(2, 1, 0, 3))

    def wtile(W):
        # [D, GD_slice] -> [p, a, d]
        return np.ascontiguousarray(W.reshape(DMT, 128, -1).transpose(1, 0, 2))

    in_maps = []
    for core in range(NCORES):
        b, g = core // 2, core % 2
        sl = slice(g * GD, (g + 1) * GD)
        in_maps.append({
            "xq": slab(q[b]).astype(dt),
            "xk": slab(k[b]).astype(dt),
            "xv": slab(v[b]).astype(dt),
            "wq": wtile(Wq[:, sl]).astype(dt),
            "wk": wtile(Wk[:, sl]).astype(dt),
            "wv": wtile(Wv[:, sl]).astype(dt),
            "wo": np.ascontiguousarray(
                Wo[sl, :].reshape(NCH, 128, D).transpose(1, 0, 2)).astype(dt),
            "bq": np.ascontiguousarray(bq[sl].reshape(NCH, 128).T),
            "bk": np.ascontiguousarray(bk[sl].reshape(NCH, 128).T),
        })
    return in_maps


def combine_outputs(results, inputs):
    Wo = np.asarray(inputs["Wo"], np.float32)
    bv = np.asarray(inputs["bv"], np.float32)
    bo = np.asarray(inputs["bo"], np.float32)
    out = np.empty((B, S, D), np.float32)
    for b in range(B):
        out[b] = results[2 * b]["out"] + results[2 * b + 1]["out"]
    out += bv @ Wo + bo
    return out


def kernel(**inputs):
    from concourse.bass_utils import run_bass_kernel_spmd
    nc = get_program()
    in_maps = make_in_maps(inputs)
    res = run_bass_kernel_spmd(nc, in_maps, list(range(NCORES)))
    return combine_outputs(res.results, inputs)


# revision 33
# speedup vs baseline: 1.1746x; 1.0043x over previous
"""Multi-head attention (B=4, S=2048, D=1024, H=16) on 8 trn2 NeuronCores.

Sharding: core = (batch b, head-group g) with b = core//2, g = core%2.
Each core handles one batch and 8 heads (512 of the 1024 d_model dims):
  - host pre-tiles query/key/value[b] and the weight slices into the exact
    SBUF layouts ([sc, 128, a, s] slabs / [128, a, d] weights) so every DMA
    is a contiguous 8KB-per-partition transfer (no strided descriptors)
  - device computes Q^T, K^T (head dims on partitions) and V (natural),
    attention with *transposed* scores S^T = K_h @ Q_h^T so softmax's
    denominator comes out of the PV matmul via a ones-column appended to V
  - output projection vs Wo[g*512:(g+1)*512, :] gives a partial [2048,1024]
  - host sums the two group partials per batch and adds bv@Wo + bo
Matmul operand dtype is MM_DT (bf16 default). PSUM accumulation and the
softmax normalization chain stay fp32.

Scheduling (the exp stream on ScalarE, 256x [128,1024] ACTIVATEs at
~1.15us each, is the pipeline's pace-setter; PE stream work is ~85-95%
of it):
  - kt is processed in blocks of 2: both kt's score matmul halves are
    interleaved with alternating 64-row PE groups so the two streams run
    concurrently on the systolic array; the previous block's PV matmuls
    follow (PV lags its exp by one block so the in-order PE queue never
    parks on an exp wait); then one 4-MM projection chunk from a global
    filler queue (whole 8-MM groups starved the exp stream).
  - prologue DMAs go in two dependency-gated waves (the DMA engines
    fair-share HBM bandwidth, so ungated transfers all land at the same
    late instant): wave1 = everything the first scores block + first PV
    needs, wave2 = Wo.
  - pair 3's last q range runs head 1 first, then head 0 as two 512-wide
    half-loops, so q tiles 8-11 project during the final attention work
    and only q tiles 12-15 remain after it.
"""

import os
import numpy as np
from contextlib import ExitStack

B = 4
S = 2048
D = 1024
H = 16
DK = 64
NCORES = 8
GH = 8          # heads per core (group)
GD = GH * DK    # 512 head dims per core
NCH = GD // 128  # 4 chunks of 128 output dims
KT = S // 128    # 16 key tiles
QC = 1024        # q chunk width for attention
NQC = S // QC    # 2
SC = 512         # s chunk width for projections
NSC = S // SC    # 4
DMT = D // 128   # 8 d_model tiles

MM_DT = os.environ.get("MM_DT", "bf16")  # "bf16" | "f32r"

_CACHE = {}


def _np_mm_dtype():
    if MM_DT == "bf16":
        import ml_dtypes
        return ml_dtypes.bfloat16
    return np.float32


def _build_program():
    import concourse.mybir as mybir
    import concourse.tile as tile
    from concourse import bacc
    from concourse.tile_rust import add_dep_helper

    f32 = mybir.dt.float32
    dmm = mybir.dt.bfloat16 if MM_DT == "bf16" else mybir.dt.float32r

    nc = bacc.Bacc("TRN2", target_bir_lowering=False, debug=False,
                   num_devices=NCORES)

    xq = nc.dram_tensor("xq", [NSC, 128, DMT, SC], dmm,
                        kind="ExternalInput").ap()
    xk = nc.dram_tensor("xk", [NSC, 128, DMT, SC], dmm,
                        kind="ExternalInput").ap()
    xv = nc.dram_tensor("xv", [NSC, 128, DMT, SC], dmm,
                        kind="ExternalInput").ap()
    wq = nc.dram_tensor("wq", [128, DMT, GD], dmm, kind="ExternalInput").ap()
    wk = nc.dram_tensor("wk", [128, DMT, GD], dmm, kind="ExternalInput").ap()
    wv = nc.dram_tensor("wv", [128, DMT, GD], dmm, kind="ExternalInput").ap()
    wo = nc.dram_tensor("wo", [128, NCH, D], dmm, kind="ExternalInput").ap()
    bq = nc.dram_tensor("bq", [128, NCH], f32, kind="ExternalInput").ap()
    bk = nc.dram_tensor("bk", [128, NCH], f32, kind="ExternalInput").ap()
    out = nc.dram_tensor("out", [S, D], f32, kind="ExternalOutput").ap()

    Exp = mybir.ActivationFunctionType.Exp

    with tile.TileContext(nc) as tc, ExitStack() as ctx:
        # ---- pools (slots are statically reserved per tag) ----
        p_qt = ctx.enter_context(tc.tile_pool(name="qt", bufs=GH))
        p_kt = ctx.enter_context(tc.tile_pool(name="kt", bufs=GH))
        p_v = ctx.enter_context(tc.tile_pool(name="v", bufs=KT))
        p_ot = ctx.enter_context(tc.tile_pool(name="ot", bufs=NCH))
        p_wvo = ctx.enter_context(tc.tile_pool(name="wvo", bufs=1))
        p_wc = ctx.enter_context(tc.tile_pool(name="wc", bufs=1))
        p_bias = ctx.enter_context(tc.tile_pool(name="bias", bufs=1))
        p_xs = ctx.enter_context(tc.tile_pool(name="xs", bufs=4))
        p_pt = ctx.enter_context(tc.tile_pool(name="pt", bufs=7))
        p_zr = ctx.enter_context(tc.tile_pool(name="zr", bufs=2))
        p_rb = ctx.enter_context(tc.tile_pool(name="rb", bufs=2))
        p_st = ctx.enter_context(tc.tile_pool(name="st", bufs=2))
        p_ov = ctx.enter_context(tc.tile_pool(name="ov", bufs=2))
        # PSUM: ps 3 slots x 2 banks + pv 1 slot x 2 banks = all 8 banks
        p_ps = ctx.enter_context(tc.tile_pool(name="ps", bufs=3, space="PSUM"))
        p_pv = ctx.enter_context(tc.tile_pool(name="pv", bufs=1, space="PSUM"))

        # ---- biases + ones (tiny, sync queue) ----
        bq_sb = p_bias.tile([128, NCH], f32, tag="bq")
        nc.sync.dma_start(out=bq_sb[:], in_=bq)
        bk_sb = p_bias.tile([128, NCH], f32, tag="bk")
        nc.sync.dma_start(out=bk_sb[:], in_=bk)
        ones_sb = p_bias.tile([128, 1], f32, tag="ones")
        nc.vector.memset(ones_sb[:], 1.0)

        # ---- prologue DMAs in two dependency-gated waves: the DMA engines
        # fair-share HBM bandwidth across everything in flight, so gate the
        # non-critical wave behind the 5MB the first scores matmul needs
        # (wk, wq, xs k0/q0/q1). V arrives in wave2; the exp stream runs
        # ~7 kt ahead of PV, which absorbs the later V start.
        wk_sb = p_wc.tile([128, DMT, GD], dmm, tag="wkc", name="wk_sb")
        i_wk = nc.gpsimd.dma_start(out=wk_sb[:], in_=wk)
        wq_sb = p_wc.tile([128, DMT, GD], dmm, tag="wqc", name="wq_sb")
        i_wq = nc.gpsimd.dma_start(out=wq_sb[:], in_=wq)
        xs_k0 = p_xs.tile([128, DMT, SC], dmm, tag="xs", name="xs_k0")
        i_k0 = nc.sync.dma_start(out=xs_k0[:], in_=xk[0])
        xs_q0 = p_xs.tile([128, DMT, SC], dmm, tag="xs", name="xs_q0")
        i_q0 = nc.sync.dma_start(out=xs_q0[:], in_=xq[0])
        xs_q1 = p_xs.tile([128, DMT, SC], dmm, tag="xs", name="xs_q1")
        i_q1 = nc.scalar.dma_start(out=xs_q1[:], in_=xq[1])
        wave1 = [i_wk, i_wq, i_k0, i_q0, i_q1]

        # V inputs gated on just {wk, k0}: they stop competing with the
        # scores-critical q transfers for HBM bandwidth, but still land
        # early enough for the first PV block
        xv_0 = p_xs.tile([128, DMT, 512], dmm, tag="xs", name="xv0")
        i_xv0 = nc.scalar.dma_start(out=xv_0[:], in_=xv[0])
        vstate = {"pre": xv_0}
        wv_sb = p_wvo.tile([128, DMT, GD], dmm, tag="wvo", name="wv_sb")
        i_wv = nc.gpsimd.dma_start(out=wv_sb[:], in_=wv)
        wave1b = [i_xv0, i_wv]

        wo_sb = p_wvo.tile([128, NCH, D], dmm, tag="wo", name="wo_sb")
        i_wo = nc.scalar.dma_start(out=wo_sb[:], in_=wo)
        wave2 = [i_wo]

        for later, earlier in (
                [(b, a) for b in wave1b for a in (i_wk, i_k0)]
                + [(w, a) for w in wave2 for a in wave1 + wave1b]):
            add_dep_helper(later.ins, earlier.ins, reason="dma wave order")

        qt_sb = [None] * GH
        kt_sb = [None] * GH
        ot_sb = [None] * NCH
        v_sb = []

        # ---- V projection: V_sb[st] = [128 s, GH, 65] (col 64 = ones) ----
        def emit_v_st(st):
            if st % 4 == 0:
                if "pre" in vstate:
                    xv_t = vstate.pop("pre")
                else:
                    xv_t = p_xs.tile([128, DMT, 512], dmm, tag="xs",
                                     name=f"xv{st}")
                    nc.sync.dma_start(out=xv_t[:], in_=xv[st // 4])
                vstate["xv"] = xv_t
            xv_t = vstate["xv"]
            sub = (st % 4) * 128
            ps = p_ps.tile([128, 1024], f32, tag="ps", name=f"psv{st}")
            for a in range(DMT):
                nc.tensor.matmul(
                    out=ps[:, 0:GD],
                    lhsT=xv_t[:, a, sub:sub + 128],
                    rhs=wv_sb[:, a, :],
                    start=(a == 0), stop=(a == DMT - 1),
                )
            vt = p_v.tile([128, GH, 65], dmm, tag="v", name=f"v{st}")
            nc.vector.tensor_copy(
                out=vt[:, :, 0:DK],
                in_=ps[:, 0:GD].rearrange("p (h d) -> p h d", h=GH),
            )
            nc.vector.tensor_copy(
                out=vt[:, :, DK:65],
                in_=ones_sb.unsqueeze(1).broadcast_to([128, GH, 1]))
            v_sb.append(vt)

        def v_filler(st):
            def emit():
                emit_v_st(st)
            return emit

        # ---- Q/K projections for head pair c; one group = one s-chunk ----
        def alloc_pair(c):
            for hh in range(2):
                hg = 2 * c + hh
                qt_sb[hg] = p_qt.tile([128, S], dmm, tag="qt", name=f"qt{hg}")
                kt_sb[hg] = p_kt.tile([128, S], dmm, tag="kt", name=f"kt{hg}")

        def proj_chunks(c, which, sc, xs_pre=None):
            """Q^T/K^T slice for heads 2c,2c+1 over s-range sc, split into
            two 4-MM chunks so filler work interleaves with the attention
            matmul stream without starving ACT. Head tile [128, S] holds
            its 64 dims twice (rows 0-63 and 64-127) so consecutive kt
            score matmuls alternate PE row groups and run concurrently."""
            src, wsb, bsb = ((xq, wq_sb, bq_sb) if which == "q"
                            else (xk, wk_sb, bk_sb))
            dsts = ([qt_sb[2 * c], qt_sb[2 * c + 1]] if which == "q"
                    else [kt_sb[2 * c], kt_sb[2 * c + 1]])
            state = {}

            def emit_a():
                if xs_pre is not None:
                    xs = xs_pre
                else:
                    xs = p_xs.tile([128, DMT, SC], dmm, tag="xs",
                                   name=f"xs{which}{c}_{sc}")
                    nc.sync.dma_start(out=xs[:], in_=src[sc])
                ps = p_ps.tile([128, 1024], f32, tag="ps",
                               name=f"psp{which}{c}_{sc}")
                for a in range(4):
                    nc.tensor.matmul(
                        out=ps[:, 0:SC],
                        lhsT=wsb[:, a, c * 128:(c + 1) * 128],
                        rhs=xs[:, a, :],
                        start=(a == 0), stop=False,
                    )
                state["xs"], state["ps"] = xs, ps

            def emit_b():
                xs, ps = state["xs"], state["ps"]
                for a in range(4, DMT):
                    nc.tensor.matmul(
                        out=ps[:, 0:SC],
                        lhsT=wsb[:, a, c * 128:(c + 1) * 128],
                        rhs=xs[:, a, :],
                        start=False, stop=(a == DMT - 1),
                    )
                s0, s1 = sc * SC, (sc + 1) * SC
                # head 2c native rows 0-63; head 2c+1 native rows 64-127
                nc.vector.tensor_scalar_add(
                    out=dsts[0][0:DK, s0:s1], in0=ps[0:DK, 0:SC],
                    scalar1=bsb[0:DK, c:c + 1])
                nc.vector.tensor_scalar_add(
                    out=dsts[1][DK:128, s0:s1], in0=ps[DK:128, 0:SC],
                    scalar1=bsb[DK:128, c:c + 1])
                # duplicate this slice into the other half right away
                # (SBUF->SBUF DMA) so scores kt for this s-range unblock
                nc.sync.dma_start(out=dsts[0][DK:128, s0:s1],
                                  in_=dsts[0][0:DK, s0:s1])
                nc.sync.dma_start(out=dsts[1][0:DK, s0:s1],
                                  in_=dsts[1][DK:128, s0:s1])
            return [emit_a, emit_b]

        # ---- output projection for one 128-row q tile, two 4-MM chunks ----
        def fin_chunks(qt_i):
            state = {}

            def emit_a():
                ps = p_ps.tile([128, 1024], f32, tag="ps", name=f"pso{qt_i}")
                for cc in range(2):
                    for half in range(2):
                        nc.tensor.matmul(
                            out=ps[:, half * 512:(half + 1) * 512],
                            lhsT=ot_sb[cc][:, qt_i * 128:(qt_i + 1) * 128],
                            rhs=wo_sb[:, cc, half * 512:(half + 1) * 512],
                            start=(cc == 0), stop=False,
                        )
                state["ps"] = ps

            def emit_b():
                ps = state["ps"]
                for cc in range(2, NCH):
                    for half in range(2):
                        nc.tensor.matmul(
                            out=ps[:, half * 512:(half + 1) * 512],
                            lhsT=ot_sb[cc][:, qt_i * 128:(qt_i + 1) * 128],
                            rhs=wo_sb[:, cc, half * 512:(half + 1) * 512],
                            start=False, stop=(cc == NCH - 1),
                        )
                st = p_st.tile([128, D], f32, tag="st", name=f"st{qt_i}")
                nc.vector.tensor_copy(out=st[:], in_=ps[:])
                nc.sync.dma_start(out=out[qt_i * 128:(qt_i + 1) * 128, :],
                                  in_=st[:])
            return [emit_a, emit_b]

        # ---- one attention loop: head hg = 2c+hh, q columns qc*QC.. ----
        def attention_loop(c, qc, hh, slots=(), fillers=None, queue=None,
                           qcap=10, qw=QC):
            """fillers: per-loop whole-group callbacks popped at `slots`
            (two per 2-kt block); queue: global 4-MM chunk queue popped
            once per block with kb <= qcap."""
            slots = set(slots)
            fillers = fillers if fillers is not None else []
            hg = 2 * c + hh
            if ot_sb[c] is None:
                ot_sb[c] = p_ot.tile([128, S], dmm, tag="ot", name=f"ot{c}")
            qb = qc * 512
            pv_ps = p_pv.tile([65, qw], f32, tag="pv",
                              name=f"pv{c}_{qc}_{hh}")
            pts = {}

            def emit_pv(kt_i):
                pt = pts.pop(kt_i)
                for half in range(qw // 512):
                    nc.tensor.matmul(
                        out=pv_ps[:, half * 512:(half + 1) * 512],
                        lhsT=v_sb[kt_i][:, hg, :],
                        rhs=pt[:, half * 512:(half + 1) * 512],
                        start=(kt_i == 0), stop=(kt_i == KT - 1),
                    )

            # kt processed in blocks of 2: both kt's score halves are
            # interleaved (alternating 64-row PE groups -> the two streams
            # run concurrently), then the PVs of the PREVIOUS block, then
            # filler chunks. PV lags its exp by one block so the in-order
            # PE stream never parks on an exp wait, and same-weight-shape
            # matmuls stay contiguous (fewer exposed LDWEIGHTS).
            for kb in range(0, KT, 2):
                pss = []
                for j in range(2):
                    pss.append(p_ps.tile([128, qw], f32, tag="ps",
                                         name=f"pss{c}_{qc}_{kb + j}_{hh}"))
                for half in range(qw // 512):
                    q0 = qb + half * 512
                    for j in range(2):
                        kt_i = kb + j
                        rg = DK * j
                        nc.tensor.matmul(
                            out=pss[j][:, half * 512:(half + 1) * 512],
                            lhsT=kt_sb[hg][rg:rg + DK,
                                           kt_i * 128:(kt_i + 1) * 128],
                            rhs=qt_sb[hg][rg:rg + DK, q0:q0 + 512],
                            start=True, stop=True,
                        )
                for j in range(2):
                    kt_i = kb + j
                    pt = p_pt.tile([128, qw], dmm, tag="pt",
                                   name=f"pt{c}_{qc}_{kt_i}_{hh}")
                    nc.scalar.activation(pt[:], pss[j][:], Exp,
                                         bias=0.0, scale=0.125)
                    pts[kt_i] = pt
                if kb >= 2:
                    emit_pv(kb - 2)
                    emit_pv(kb - 1)
                if fillers:
                    if kb in slots:
                        fillers.pop(0)()
                    if kb + 1 in slots and fillers:
                        fillers.pop(0)()
                elif queue and kb <= qcap:
                    queue.pop(0)()
            emit_pv(KT - 2)
            emit_pv(KT - 1)
            # evict PV psum right away to release its bank pair
            ovt = p_ov.tile([65, qw], f32, tag="ov", name=f"ov{c}_{qc}_{hh}")
            nc.vector.tensor_copy(out=ovt[:], in_=pv_ps[:])
            # normalize off the critical path:
            # O^T = PV[0:64] * broadcast(1 / PV[64])
            zs = p_zr.tile([DK, qw // DK], f32, tag="zs",
                           name=f"zs{c}_{qc}_{hh}")
            nc.sync.dma_start(out=zs[:], in_=ovt[DK:DK + 1, :])
            nc.vector.reciprocal(out=zs[:], in_=zs[:])
            zr = p_zr.tile([1, qw], f32, tag="zr", name=f"zr{c}_{qc}_{hh}")
            nc.sync.dma_start(out=zr[:], in_=zs[:])
            rb = p_rb.tile([DK, qw], f32, tag="rb", name=f"rb{c}_{qc}_{hh}")
            nc.gpsimd.partition_broadcast(rb[:], zr[:], channels=DK)
            if hh == 0:
                nc.vector.tensor_mul(
                    out=ot_sb[c][0:DK, qb:qb + qw],
                    in0=ovt[0:DK, :], in1=rb[:])
            else:
                tmp = p_rb.tile([DK, qw], dmm, tag="rb", name=f"tmp{c}_{qc}")
                nc.vector.tensor_mul(out=tmp[:], in0=ovt[0:DK, :], in1=rb[:])
                nc.sync.dma_start(
                    out=ot_sb[c][DK:128, qb:qb + qw],
                    in_=tmp[:])

        # ================= emission =================
        # prologue: pair-0 Q/K s-chunks 0/1 + V chunk 0 (inputs pre-issued
        # in the DMA waves above)
        alloc_pair(0)
        for e in proj_chunks(0, "k", 0, xs_pre=xs_k0):
            e()
        for e in proj_chunks(0, "q", 0, xs_pre=xs_q0):
            e()
        for e in proj_chunks(0, "q", 1, xs_pre=xs_q1):
            e()

        alloc_pair(1)
        alloc_pair(2)
        alloc_pair(3)

        # loop (0,0,0) consumes v0..v15 + k1..k3 just-in-time as whole
        # groups (it is PE-bound regardless); everything downstream drains
        # from the global 4-MM chunk queue, one chunk per odd kt
        k01 = proj_chunks(0, "k", 1)
        k02 = proj_chunks(0, "k", 2)
        k03 = proj_chunks(0, "k", 3)
        f000 = [v_filler(0),
                lambda: (v_filler(1)(), k01[0](), k01[1]()),
                v_filler(2),
                lambda: (v_filler(3)(), k02[0](), k02[1]()),
                v_filler(4),
                lambda: (v_filler(5)(), k03[0](), k03[1]()),
                v_filler(6), v_filler(7), v_filler(8), v_filler(9),
                v_filler(10), v_filler(11), v_filler(12), v_filler(13),
                v_filler(14), v_filler(15)]

        FQ = []
        for cc, w, sc in [(0, "q", 2), (0, "q", 3),
                          (1, "k", 0), (1, "q", 0), (1, "q", 1), (1, "k", 1),
                          (1, "k", 2), (1, "q", 2), (1, "k", 3), (1, "q", 3),
                          (2, "k", 0), (2, "q", 0), (2, "q", 1), (2, "k", 1),
                          (2, "k", 2), (2, "q", 2), (2, "k", 3), (2, "q", 3),
                          (3, "k", 0), (3, "q", 0), (3, "q", 1), (3, "k", 1),
                          (3, "k", 2), (3, "q", 2), (3, "k", 3), (3, "q", 3)]:
            FQ.extend(proj_chunks(cc, w, sc))

        FIN = []
        for qt_i in range(12):
            FIN.extend(fin_chunks(qt_i))

        for c in range(NCH):
            for qc in range(NQC):
                for hh in range(2):
                    if (c, qc, hh) == (0, 0, 0):
                        attention_loop(c, 0, 0, slots=range(0, 16),
                                       fillers=f000)
                    elif c == 3 and qc == 1:
                        pass  # emitted below in tail-friendly order
                    else:
                        attention_loop(c, qc * 2, hh, queue=FQ, qcap=10)
        # pair-3 qc1: head 1 first (full width), then head 0 as two
        # 512-wide half loops -> q tiles 8-11 finish one half-loop early
        # and their output projection overlaps the last attention work;
        # only q tiles 12-15 remain after all attention
        attention_loop(3, 2, 1, queue=FIN, qcap=14)
        attention_loop(3, 2, 0, queue=FIN, qcap=14, qw=512)
        attention_loop(3, 3, 0, queue=FIN, qcap=14, qw=512)
        for e in FIN:
            e()
        for qt_i in range(12, KT):
            a, b2 = fin_chunks(qt_i)
            a()
            b2()

    nc.compile()
    return nc


def get_program():
    if "nc" not in _CACHE:
        _CACHE["nc"] = _build_program()
    return _CACHE["nc"]


def make_in_maps(inputs):
    dt = _np_mm_dtype()
    q = np.asarray(inputs["query"], np.float32)
    k = np.asarray(inputs["key"], np.float32)
    v = np.asarray(inputs["value"], np.float32)
    Wq = np.asarray(inputs["Wq"], np.float32)
    Wk = np.asarray(inputs["Wk"], np.float32)
    Wv = np.asarray(inputs["Wv"], np.float32)
    Wo = np.asarray(inputs["Wo"], np.float32)
    bq = np.asarray(inputs["bq"], np.float32)
    bk = np.asarray(inputs["bk"], np.float32)

    def slab(x):
        # [S, D] -> x.T [D, S] -> [sc, p, a, s] contiguous slabs
        return np.ascontiguousarray(
            x.T.reshape(DMT, 128, NSC, SC).transpose(2, 1, 0, 3))

    def wtile(W):
        # [D, GD_slice] -> [p, a, d]
        return np.ascontiguousarray(W.reshape(DMT, 128, -1).transpose(1, 0, 2))

    in_maps = []
    for core in range(NCORES):
        b, g = core // 2, core % 2
        sl = slice(g * GD, (g + 1) * GD)
        in_maps.append({
            "xq": slab(q[b]).astype(dt),
            "xk": slab(k[b]).astype(dt),
            "xv": slab(v[b]).astype(dt),
            "wq": wtile(Wq[:, sl]).astype(dt),
            "wk": wtile(Wk[:, sl]).astype(dt),
            "wv": wtile(Wv[:, sl]).astype(dt),
            "wo": np.ascontiguousarray(
                Wo[sl, :].reshape(NCH, 128, D).transpose(1, 0, 2)).astype(dt),
            "bq": np.ascontiguousarray(bq[sl].reshape(NCH, 128).T),
            "bk": np.ascontiguousarray(bk[sl].reshape(NCH, 128).T),
        })
    return in_maps


def combine_outputs(results, inputs):
    Wo = np.asarray(inputs["Wo"], np.float32)
    bv = np.asarray(inputs["bv"], np.float32)
    bo = np.asarray(inputs["bo"], np.float32)
    out = np.empty((B, S, D), np.float32)
    for b in range(B):
        out[b] = results[2 * b]["out"] + results[2 * b + 1]["out"]
    out += bv @ Wo + bo
    return out


def kernel(**inputs):
    from concourse.bass_utils import run_bass_kernel_spmd
    nc = get_program()
    in_maps = make_in_maps(inputs)
    res = run_bass_kernel_spmd(nc, in_maps, list(range(NCORES)))
    return combine_outputs(res.results, inputs)
